# revision 16
# baseline (speedup 1.0000x reference)
"""Trainium2 Bass kernel for a rate-1/2, constraint-length-3 feedforward
convolutional encoder (generator polynomials "101" and "111", MSB-first).

The trellis scan in the reference collapses to elementwise XORs of shifted
input bits (zero initial state):

    out0[t] = u[t] ^ u[t-2]            (poly "101")
    out1[t] = u[t] ^ u[t-1] ^ u[t-2]   (poly "111")

with the codeword interleaved time-major: y[:, 2t] = out0[t], y[:, 2t+1] = out1[t].

All values are exactly 0/1, so the kernel moves single bytes instead of
f32 (host casts f32<->u8, exact): 6 MiB of HBM traffic per core instead
of 24 MiB — the binding roofline at ~360 GB/s/core.

Compute runs as two custom DVE ops (registered at import time into
concourse's dve_ops table) over uint16 *byte-pair* views of the input,
v = x[2i] + 256*x[2i+1] and w = x[2i-2] + 256*x[2i-1]:

    even op: out16[2i]   = e0 + 256*(e0^c1)   (bytes E[2i], O[2i])
    odd  op: out16[2i+1] = e1 + 256*(e1^a0)   (bytes E[2i+1], O[2i+1])

The DVE custom-op datapath is fp32, so the bit logic uses exact {0,1}
arithmetic: with d = |v-w| ∈ {0,1,255,256,257},
e0 = parity(d) = (d!=0)(d!=256), e1 = (d>=255), a0 = parity(v),
c1 = (w>=256). Each op emits two interleaved output bytes per cycle per
lane, so the DVE (~18.6 us/core) is the critical path right at the DMA
roofline; each fits the 8-stage DVE pipeline exactly (verified bit-exact
on HW).

Structure: raw bass (no TileContext) with manual semaphores; per-group
chains in-DMA -> even/odd ops -> out-DMA over dedicated SBUF slots.
The host prepends the 2-byte zero encoder state to every row so input
DMAs are fully contiguous (one descriptor per partition); interior
groups pack 2 rows per partition (4.1 KiB reads / 8 KiB writes). Small
edge groups + a split first DMA shorten the un-overlapped lead-in/tail.

Sharding: pure data parallel over the batch dim across 8 NeuronCores.
"""

import contextlib

import numpy as np

N_CORES = 8
B, K = 8192, 2048
N_OUT = 2
SHARD_B = B // N_CORES   # 1024 codewords per core
P = 128                  # SBUF partitions
CHUNK = K + 2            # 2050 B: [0, 0, row] (host-prepended zero state)

GROUP_RPP = [1, 2, 2, 1, 1, 1]  # rows-per-partition per group

_compiled = {}


def _register_ops():
    import concourse.dve_ops as dve_ops
    from concourse.dve_table_gen import dve_ver_for
    from concourse.dve_spec import (
        Spec, Src0, Src1, C0, C1, AluOp, Bin, Zero, lower, _has_src1,
    )
    from concourse.dve_uop import DveOpSpec

    def register(name, spec):
        if name in dve_ops._SUB_OPCODE_FOR_NAME:
            return next(op for op in dve_ops.OPS if op.name == name)
        row = dve_ops._CUSTOM_DVE_ROW_BASE + len(dve_ops.OPS)
        assert row < 0x20, "custom-DVE opcode row overflow"
        dve_ops._SUB_OPCODE_FOR_NAME[name] = row
        ver = dve_ver_for("TRN2")
        s = DveOpSpec(name=name, opcode=row, uops=lower(spec, ver=ver),
                      rd1_en=_has_src1(spec))
        op = dve_ops.DveOp(name, spec, subdim=False,
                           uops_sha={ver: s.sha(ver)})
        dve_ops.OPS.append(op)
        dve_ops.CUSTOM_DVE_SPECS[name] = spec
        return op

    # C0=255, C1=256. C1 doubles as byte threshold and interleave
    # multiplier (imm2 is unavailable with a 2-free-dim src1).
    d = Bin(AluOp.ABSOLUTE_DIFF, Src0, Src1)
    e0 = Bin(AluOp.IS_NE, d, Zero) * Bin(AluOp.IS_NE, d, C1)
    c1 = Src1 >= C1
    even_body = e0 + Bin(AluOp.IS_NE, e0, c1) * C1

    e1 = d >= C0
    a0 = Bin(AluOp.IS_NE, Src0, Zero) * Bin(AluOp.IS_NE, Src0, C1)
    odd_body = e1 + Bin(AluOp.IS_NE, e1, a0) * C1

    def ref_even(in0, in1, s0, s1, imm2):
        v, w = in0.astype(np.int64), in1.astype(np.int64)
        dd = np.abs(v - w)
        e = ((dd != 0) & (dd != 256)).astype(np.int64)
        o = (e != (w >= 256)).astype(np.int64)
        return (e + 256 * o).astype(np.float32)

    def ref_odd(in0, in1, s0, s1, imm2):
        v, w = in0.astype(np.int64), in1.astype(np.int64)
        dd = np.abs(v - w)
        e = (dd >= 255).astype(np.int64)
        o = (e != ((v != 0) & (v != 256))).astype(np.int64)
        return (e + 256 * o).astype(np.float32)

    ev = register("CONV_ENC_EVEN", Spec(body=even_body, reference=ref_even))
    od = register("CONV_ENC_ODD", Spec(body=odd_body, reference=ref_odd))
    return ev, od


def _build_nc():
    from concourse import bacc, mybir

    ev, od = _register_ops()

    nc = bacc.Bacc(
        "TRN2",
        target_bir_lowering=False,
        debug=False,
        enable_asserts=False,
    )
    x = nc.dram_tensor(
        "x", [SHARD_B, CHUNK], mybir.dt.uint8, kind="ExternalInput"
    ).ap()
    y = nc.dram_tensor(
        "y", [SHARD_B, N_OUT * K], mybir.dt.uint8, kind="ExternalOutput"
    ).ap()

    H = K // 2
    n_groups = len(GROUP_RPP)
    assert sum(r * P for r in GROUP_RPP) == SHARD_B

    with contextlib.ExitStack() as stack:
        slots = [
            stack.enter_context(
                nc.sbuf_tensor(f"z{g}", [P, rpp * CHUNK + 4], mybir.dt.uint8)
            )
            for g, rpp in enumerate(GROUP_RPP)
        ]
        wtiles = [
            stack.enter_context(
                nc.sbuf_tensor(f"w{g}", [P, rpp * N_OUT * K], mybir.dt.uint8)
            )
            for g, rpp in enumerate(GROUP_RPP)
        ]
        s_in = stack.enter_context(nc.semaphore())
        s_ve = stack.enter_context(nc.semaphore())
        s_out = stack.enter_context(nc.semaphore())
        block = stack.enter_context(nc.Block())

        row_starts = []
        r0 = 0
        for rpp in GROUP_RPP:
            row_starts.append(r0)
            r0 += rpp * P

        # Group 0's input lands as two column-halves so the first custom op
        # starts after only ~half of the first DMA (compute for u16 pairs
        # [0, HSPLIT) needs input bytes [0, 2*HSPLIT+2)).
        HSPLIT = H // 2

        @block.sync
        def _(sync):
            for g, rpp in enumerate(GROUP_RPP):
                rows = slice(row_starts[g], row_starts[g] + rpp * P)
                src = x[rows, :].rearrange("(p j) k -> p (j k)", j=rpp)
                if g == 0:
                    cut = 2 * HSPLIT + 2
                    sync.dma_start(
                        slots[0][:, 0:cut], src[:, 0:cut]
                    ).then_inc(s_in, 16)
                    sync.dma_start(
                        slots[0][:, cut:CHUNK], src[:, cut:CHUNK]
                    ).then_inc(s_in, 16)
                else:
                    sync.dma_start(
                        slots[g][:, 0 : rpp * CHUNK], src
                    ).then_inc(s_in, 16)
            sync.wait_ge(s_out, 16 * (n_groups + 1))

        @block.vector
        def _(vector):
            sem_base = 0
            for g, rpp in enumerate(GROUP_RPP):
                z16 = slots[g][:].bitcast(mybir.dt.uint16)
                a16 = (
                    z16[:, 1 : 1 + rpp * (CHUNK // 2)]
                    .rearrange("p (j n) -> p j n", j=rpp)[:, :, 0:H]
                )
                c16 = (
                    z16[:, 0 : rpp * (CHUNK // 2)]
                    .rearrange("p (j n) -> p j n", j=rpp)[:, :, 0:H]
                )
                w16 = (
                    wtiles[g][:].bitcast(mybir.dt.uint16)
                    .rearrange("p (j n) -> p j n", j=rpp)
                )
                if g == 0:
                    vector.wait_ge(s_in, 16)
                    vector._custom_dve(
                        ev, out=w16[:, :, 0 : 2 * HSPLIT : 2],
                        in0=a16[:, :, 0:HSPLIT], in1=c16[:, :, 0:HSPLIT],
                        s0=255.0, s1=256.0,
                    )
                    vector._custom_dve(
                        od, out=w16[:, :, 1 : 2 * HSPLIT : 2],
                        in0=a16[:, :, 0:HSPLIT], in1=c16[:, :, 0:HSPLIT],
                        s0=255.0, s1=256.0,
                    )
                    vector.wait_ge(s_in, 32)
                    vector._custom_dve(
                        ev, out=w16[:, :, 2 * HSPLIT : N_OUT * H : 2],
                        in0=a16[:, :, HSPLIT:H], in1=c16[:, :, HSPLIT:H],
                        s0=255.0, s1=256.0,
                    )
                    vector._custom_dve(
                        od, out=w16[:, :, 2 * HSPLIT + 1 : N_OUT * H : 2],
                        in0=a16[:, :, HSPLIT:H], in1=c16[:, :, HSPLIT:H],
                        s0=255.0, s1=256.0,
                    ).then_inc(s_ve, 1)
                    sem_base = 32
                    continue
                vector.wait_ge(s_in, sem_base + 16 * g)
                if g == n_groups - 1:
                    # last group in two column-halves so its first output
                    # half ships while the second is still computing
                    vector._custom_dve(
                        ev, out=w16[:, :, 0 : H : 2],
                        in0=a16[:, :, 0 : H // 2], in1=c16[:, :, 0 : H // 2],
                        s0=255.0, s1=256.0,
                    )
                    vector._custom_dve(
                        od, out=w16[:, :, 1 : H : 2],
                        in0=a16[:, :, 0 : H // 2], in1=c16[:, :, 0 : H // 2],
                        s0=255.0, s1=256.0,
                    ).then_inc(s_ve, 1)
                    vector._custom_dve(
                        ev, out=w16[:, :, H : N_OUT * H : 2],
                        in0=a16[:, :, H // 2 : H], in1=c16[:, :, H // 2 : H],
                        s0=255.0, s1=256.0,
                    )
                    vector._custom_dve(
                        od, out=w16[:, :, H + 1 : N_OUT * H : 2],
                        in0=a16[:, :, H // 2 : H], in1=c16[:, :, H // 2 : H],
                        s0=255.0, s1=256.0,
                    ).then_inc(s_ve, 1)
                    continue
                vector._custom_dve(
                    ev, out=w16[:, :, 0 : N_OUT * H : 2], in0=a16, in1=c16,
                    s0=255.0, s1=256.0,
                )
                vector._custom_dve(
                    od, out=w16[:, :, 1 : N_OUT * H : 2], in0=a16, in1=c16,
                    s0=255.0, s1=256.0,
                ).then_inc(s_ve, 1)

        @block.gpsimd
        def _(gp):
            last = n_groups - 1
            for g, rpp in enumerate(GROUP_RPP):
                rows = slice(row_starts[g], row_starts[g] + rpp * P)
                dsty = y[rows, :].rearrange("(p j) k -> p (j k)", j=rpp)
                if g == last:
                    # Two column-halves: the first half ships while the DVE
                    # still computes the second, shrinking the un-overlapped
                    # final DMA.
                    gp.wait_ge(s_ve, g + 1)
                    gp.dma_start(
                        dsty[:, 0 : N_OUT * K // 2],
                        wtiles[g][:, 0 : N_OUT * K // 2],
                    ).then_inc(s_out, 16)
                    gp.wait_ge(s_ve, g + 2)
                    gp.dma_start(
                        dsty[:, N_OUT * K // 2 : N_OUT * K],
                        wtiles[g][:, N_OUT * K // 2 : N_OUT * K],
                    ).then_inc(s_out, 16)
                else:
                    gp.wait_ge(s_ve, g + 1)
                    gp.dma_start(dsty, wtiles[g][:]).then_inc(s_out, 16)

    nc.compile()
    return nc


def _get_nc():
    if "nc" not in _compiled:
        _compiled["nc"] = _build_nc()
    return _compiled["nc"]


def _prep_in_maps(x_full):
    """f32 {0,1} input -> per-core uint8 arrays with the 2-byte zero
    encoder state prepended to every row."""
    assert x_full.shape == (B, K), x_full.shape
    xp = np.zeros((B, CHUNK), np.uint8)
    xp[:, 2:] = x_full
    return [
        {"x": xp[i * SHARD_B : (i + 1) * SHARD_B]} for i in range(N_CORES)
    ]


def kernel(**inputs) -> np.ndarray:
    from concourse.bass_utils import run_bass_kernel_spmd

    nc = _get_nc()
    in_maps = _prep_in_maps(np.asarray(inputs["inputs"]))
    res = run_bass_kernel_spmd(nc, in_maps, core_ids=list(range(N_CORES)))
    out = np.concatenate([r["y"] for r in res.results], axis=0)
    return out.astype(np.float32)
